# revision 71
# baseline (speedup 1.0000x reference)
"""GCN encoder (2-layer, BN, residual) on 8 Trainium2 NeuronCores.

Sharding: nodes partitioned contiguously across 8 cores (6250 each). Edges
bucketed by dst shard on host (integer-only preprocessing: bucket/sort/pad
edge indices, degree counts via bincount). All float math runs on device:

  - layer-1 gather table: host-fed replicated bf16 copy of x (pure dtype
    conversion; no x16 AllGather head). The d_out=rsqrt(outdeg) source norm
    is applied on device to gathered chunks from a host-shipped per-edge
    integer degree image.
  - SpMM: dma_gather of 128-edge chunks (rows->partitions) + one-hot selector
    matmul on PE accumulating m^T[feat, dst] in PSUM; selector built on DVE
    from iota==slot compare (exact 0/1 entries)
  - d_in applied via a broadcast matrix during PSUM evacuation
  - W matmul with W as the stationary operand keeps the [feat, dst] layout so
    BN (per-feature affine) uses per-partition ACT scale/bias + fused ReLU
  - BN stats: per-core partial sums + 1KB AllReduce
  - layer-2 table: pre-BN hpre tiles are transposed during layer 1 (hidden
    under the gather drain); BN+relu+d_out apply post-AllReduce in the
    transposed layout, shortening the critical path into the h16 AllGather

Performance notes (HW-profiled): the kernel is bound by the SWDGE gather
drain - 16 SDMA engines x 4 queue rings, ~1 outstanding 256B random HBM
read per (engine, ring), ~70-80 GB/s aggregate for 57.8 MB of gathered
rows. Q7 descriptor generation, PE matmuls, DVE selector builds and the
collectives all hide underneath it. single_packet=True batches each call's
48 descriptors per engine into one packet (+9%). Gather calls >768 idxs
overflow the 128-desc/engine SWDGE ring and crash the device, as does any
use of the ucode's trailing-negative-idx trim path.
"""

import sys

sys.path.insert(0, "/opt/trn_rl_repo")

import ml_dtypes
import numpy as np

_BF16 = ml_dtypes.bfloat16

P = 128
N_CORES = 8
EPS = 1e-5

import os

# compute dtype for gather tables / selectors / segment matmul
# bfloat16: the PE runs fp16 matmuls on the slow (fp32-mode) path (~314ns per
# 128x128) but bf16 on the fast path (~56ns); bf16's 8-bit mantissa is fine
# for the 2e-2 gate (measured ~7e-4 with fp16 -> ~6e-3 expected).
_F16 = os.environ.get("GCN_F16", "bfloat16")
_SORT = os.environ.get("GCN_SORT", "1") == "1"
# trailing -1 idx trimming ("1") crashes the device (tested in isolation, on
# a healthy device); "2" (whole-128-chunk trims, non-empty calls) also
# crashes. The gather ucode's trailing-negative trim path is broken on this
# build - must stay "0".
_NEGPAD = os.environ.get("GCN_NEGPAD", "0")

# dma_gather tuning (device crashes observed for very large single calls).
# single_packet=True lets each SDMA engine drain a call's descriptors as one
# packet (48 descs <= 64-desc packet ceiling at 768 idxs): ~80 vs ~71 GB/s.
GATHER_SINGLE_PACKET = os.environ.get("GCN_SP", "1") == "1"
GATHER_MAX_IDX = int(os.environ.get("GCN_MAXIDX", "768"))

_NP16 = _BF16 if _F16 == "bfloat16" else np.float16
DMA_SCRATCH = 32768  # per-partition SWDGE descriptor-ring carveout
N_SWDGE_QUEUES = 4  # each queue runs on its own Q7 core pair -> parallel desc-gen


def _cdiv(a, b):
    return -(-a // b)


# ---------------------------------------------------------------------------
# host-side integer preprocessing (indices only; no float arithmetic on data)
# ---------------------------------------------------------------------------


def _wrap_idx_image(idx_list):
    """int16 index list (len % 16 == 0) -> [128, len/16] SBUF image.

    dma_gather reads idx i from partition i%16, free slot i//16; the 16-row
    pattern must be replicated 8x across the 128 partitions (one per Q7 core).
    """
    n = idx_list.shape[0]
    assert n % 16 == 0
    img16 = idx_list.reshape(n // 16, 16).T  # [16, n/16]
    return np.tile(img16, (8, 1)).astype(np.int16)  # [128, n/16]


def _host_prep(src, dst, n_nodes):
    """Bucket edges by (dst shard, dst tile, src half); pad to uniform chunk
    capacities so all 8 cores run one identical program."""
    NC = N_CORES
    SH = n_nodes // NC
    assert SH * NC == n_nodes
    T = _cdiv(SH, P)
    # NOTE: splitting the x16/h16 AllGathers into two half-table AllGathers
    # (so A gathers start while the B half is on the wire) was tried and
    # REGRESSED 926us -> 1065us: half-size AllGathers cost 31+52us (vs 57us
    # for the full one - large fixed component), they serialize on the CC
    # rings, and the AllGather traffic contends with the concurrent gather
    # drain (drain-active ballooned 583us -> 730us). Keep single AllGathers.
    SPLIT = n_nodes // 2
    assert SPLIT < 32768 and (n_nodes - SPLIT) <= 32768

    src = np.asarray(src, np.int64)
    dst = np.asarray(dst, np.int64)

    per_core = []
    CA = CB = 1
    for k in range(NC):
        m = (dst >= k * SH) & (dst < (k + 1) * SH)
        s = src[m]
        dl = dst[m] - k * SH
        t_idx = dl // P
        slot = dl % P
        half = (s >= SPLIT).astype(np.int64)
        idxval = np.where(half == 1, s - SPLIT, s)
        per_core.append((t_idx, half, idxval, slot))
        for t in range(T):
            tm = t_idx == t
            na = int(np.count_nonzero(tm & (half == 0)))
            nb = int(np.count_nonzero(tm & (half == 1)))
            CA = max(CA, _cdiv(na, P))
            CB = max(CB, _cdiv(nb, P))

    n_chunks = T * (CA + CB)
    pairs = [(2 * b, min(2 * b + 1, T - 1)) for b in range(_cdiv(T, 2))]

    # global out-degrees (integer): shipped per-edge so the device can apply
    # the d_out=rsqrt(deg) source norm to gathered layer-1 rows (the x table
    # is fed replicated+uncast-normalized, killing the x16 AllGather head)
    outdeg_g = np.bincount(src, minlength=n_nodes).astype(np.int64)

    cores = []
    for k in range(NC):
        t_idx, half, idxval, slot = per_core[k]
        m = (dst >= k * SH) & (dst < (k + 1) * SH)
        s_orig = src[m]
        A_idx = np.zeros((T, CA * P), np.int16)
        B_idx = np.zeros((T, CB * P), np.int16)
        n_used = np.zeros((T, 2), np.int64)
        # pad slot = 255: one-hot (iota==slot) never fires -> zero column
        slots = np.full((n_chunks, P), 255.0, _NP16)
        # pad deg = 1 -> rsqrt = 1.0 (pads are zeroed by the selector anyway)
        degs = np.ones((n_chunks, P), np.int16)
        for t in range(T):
            tm = t_idx == t
            for h, (Cc, buf) in enumerate(((CA, A_idx), (CB, B_idx))):
                hm = tm & (half == h)
                iv = idxval[hm]
                sl = slot[hm]
                dv = outdeg_g[s_orig[hm]]
                # sort by src row: monotone gather addresses within a call
                # give the SDMA engines HBM locality (drain is latency-bound)
                if _SORT:
                    order = np.argsort(iv, kind="stable")
                    iv = iv[order]
                    sl = sl[order]
                    dv = dv[order]
                n = iv.shape[0]
                n_used[t, h] = n
                buf[t, :n] = iv.astype(np.int16)
                base = t * (CA + CB) + (0 if h == 0 else CA)
                for c in range(Cc):
                    lo, hi = c * P, min((c + 1) * P, n)
                    if hi > lo:
                        slots[base + c, : hi - lo] = sl[lo:hi].astype(_NP16)
                        degs[base + c, : hi - lo] = dv[lo:hi].astype(np.int16)

        # per-edge degree image reordered to gather-buffer column order so
        # the on-device d_out scaling is ONE DVE op per gather buffer
        NCHT2 = CA + CB
        degC = np.ones((len(pairs) * 2 * NCHT2, P), np.int16)
        colc = 0
        for t0, t1 in pairs:
            tl = [t0] if t0 == t1 else [t0, t1]
            for h, Cc in ((0, CA), (1, CB)):
                for ti, t in enumerate(tl):
                    base = t * NCHT2 + (0 if h == 0 else CA)
                    degC[colc + ti * Cc : colc + (ti + 1) * Cc] = degs[
                        base : base + Cc
                    ]
                colc += 2 * Cc

        # gather-call index images: one A call + one B call per tile pair
        imgs = []
        offs_a, offs_b = [], []
        col = 0
        for t0, t1 in pairs:
            tl = [t0] if t0 == t1 else [t0, t1]
            for h, (Cc, buf, offs) in enumerate(
                ((CA, A_idx, offs_a), (CB, B_idx, offs_b))
            ):
                lst = np.concatenate([buf[t] for t in tl])
                # trailing pads -> -1: the gather ucode trims trailing
                # negative idxs before descriptor generation (non-trailing
                # pads must stay 0 - negative mid-list gathers garbage addrs)
                tail_pad = Cc * P - int(n_used[tl[-1], h])
                if _NEGPAD == "2":
                    # safer variant: trim only whole 128-idx chunks and keep
                    # every gather call non-empty (>=128 real+pad idxs)
                    for c0 in range(0, len(tl) * Cc * P, GATHER_MAX_IDX):
                        c1 = min(c0 + GATHER_MAX_IDX, len(tl) * Cc * P)
                        t2 = min((tail_pad // P) * P, c1 - c0 - P)
                        if c1 == lst.shape[0] and t2 > 0:
                            lst[c1 - t2 : c1] = -1
                elif _NEGPAD == "1" and tail_pad > 0:
                    lst[lst.shape[0] - tail_pad :] = -1
                img = _wrap_idx_image(lst)
                offs.append((col, img.shape[1], len(tl) * Cc * P))
                col += img.shape[1]
                imgs.append(img)
        idx_img = np.concatenate(imgs, axis=1)  # [128, col]

        # degree counts (integers), tile-column layout [P, T], pad rows deg=1
        outdeg = np.bincount(src, minlength=n_nodes).astype(np.int64)
        indeg = np.bincount(dst, minlength=n_nodes).astype(np.int64)
        mine = slice(k * SH, (k + 1) * SH)

        def _cols(d):
            v = np.ones(T * P, np.float32)
            v[:SH] = d[mine].astype(np.float32)
            return v.reshape(T, P).T.copy()  # [P, T]

        cores.append(
            dict(
                idx_img=idx_img,
                slotT=slots.T.copy(),  # [P, n_chunks] fp16
                degT=degC.T.copy(),  # [P, NCC] int16 per-edge out-degree
                deg_out=_cols(outdeg),
                deg_in=_cols(indeg),
                offs_a=offs_a,
                offs_b=offs_b,
            )
        )

    meta = dict(
        SH=SH,
        T=T,
        SPLIT=SPLIT,
        CA=CA,
        CB=CB,
        n_chunks=n_chunks,
        pairs=pairs,
        idx_cols=cores[0]["idx_img"].shape[1],
        n_nodes=n_nodes,
        # call offsets are identical across cores by construction
        offs_a=cores[0]["offs_a"],
        offs_b=cores[0]["offs_b"],
    )
    for c in cores[1:]:
        assert c["offs_a"] == meta["offs_a"] and c["offs_b"] == meta["offs_b"]
        assert c["idx_img"].shape == cores[0]["idx_img"].shape
    return meta, cores


# ---------------------------------------------------------------------------
# device program (identical on all cores; all data-dependence through SBUF)
# ---------------------------------------------------------------------------


def _build_program(meta):
    import concourse.bacc as bacc
    import concourse.bass as bass
    import concourse.tile as tile
    from concourse import mybir
    from concourse.masks import make_identity

    f32 = mybir.dt.float32
    f16 = getattr(mybir.dt, _F16)
    Alu = mybir.AluOpType
    Act = mybir.ActivationFunctionType

    SH, T, SPLIT = meta["SH"], meta["T"], meta["SPLIT"]
    CA, CB = meta["CA"], meta["CB"]
    NCH = meta["n_chunks"]
    NN = meta["n_nodes"]
    pairs = meta["pairs"]
    rows_of = lambda t: min(P, SH - t * P)

    nc = bacc.Bacc(
        "TRN2",
        target_bir_lowering=False,
        debug=False,
        num_devices=N_CORES,
        dynamic_dma_scratch_size=DMA_SCRATCH,
        num_swdge_queues=N_SWDGE_QUEUES,
    )

    # ---- I/O -------------------------------------------------------------
    SHP = T * P  # shard rows padded to a tile multiple
    x16_full = nc.dram_tensor("x16_rep", [NN, P], f16, kind="ExternalInput")
    W1_t = nc.dram_tensor("W1", [P, P], f32, kind="ExternalInput")
    W2_t = nc.dram_tensor("W2", [P, P], f32, kind="ExternalInput")
    gm1 = nc.dram_tensor("gamma1", [P, 1], f32, kind="ExternalInput")
    bt1 = nc.dram_tensor("beta1", [P, 1], f32, kind="ExternalInput")
    gm2 = nc.dram_tensor("gamma2", [P, 1], f32, kind="ExternalInput")
    bt2 = nc.dram_tensor("beta2", [P, 1], f32, kind="ExternalInput")
    iota_t = nc.dram_tensor("iota", [P, P], f16, kind="ExternalInput")
    idx_t = nc.dram_tensor("idx_img", [P, meta["idx_cols"]], mybir.dt.int16,
                           kind="ExternalInput")
    slot_t = nc.dram_tensor("slotT", [P, NCH], f16, kind="ExternalInput")
    NCC = len(meta["pairs"]) * 2 * (CA + CB)
    degT_t = nc.dram_tensor("degT", [P, NCC], mybir.dt.int16, kind="ExternalInput")
    dego_t = nc.dram_tensor("deg_out", [P, T], f32, kind="ExternalInput")
    degi_t = nc.dram_tensor("deg_in", [P, T], f32, kind="ExternalInput")
    out_t = nc.dram_tensor("out", [SHP, P], f32, kind="ExternalOutput")

    with tile.TileContext(nc) as tc:
        with (
            tc.tile_pool(name="cst", bufs=1) as cst,
            tc.tile_pool(name="big", bufs=1) as big,
            tc.tile_pool(name="gat", bufs=3) as gat,
            tc.tile_pool(name="wrk", bufs=3) as wrk,
            tc.tile_pool(name="ps", bufs=2, space="PSUM") as ps,
            tc.tile_pool(name="dram", bufs=1, space="DRAM") as dram,
        ):
            # ---- degree normalizers first: the x16 table cast + AllGather
            # is the serial head of the kernel, so issue it before the bulk
            # static-data loads.
            d_out = cst.tile([P, T], f32)
            d_in = cst.tile([P, T], f32)
            for deg_dram, d_sb in ((dego_t, d_out), (degi_t, d_in)):
                raw = wrk.tile([P, T], f32, tag="degraw")
                nc.sync.dma_start(raw[:], deg_dram[:])
                nc.vector.tensor_scalar_max(raw[:], raw[:], 1.0)
                nc.scalar.sqrt(raw[:], raw[:])
                nc.vector.reciprocal(d_sb[:], raw[:])

            # ---- layer-1 gather table is the replicated host-fed x16; the
            # d_out source norm is applied per gathered chunk from the
            # per-edge integer degree image (no x16 AllGather head at all).
            degT_sb = cst.tile([P, NCC], mybir.dt.int16)
            nc.sync.dma_start(degT_sb[:], degT_t[:])
            dsc = cst.tile([P, NCC], f32)
            nc.vector.tensor_copy(dsc[:], degT_sb[:])
            nc.scalar.sqrt(dsc[:], dsc[:])
            nc.vector.reciprocal(dsc[:], dsc[:])
            dsc16 = cst.tile([P, NCC], f16)
            nc.vector.tensor_copy(dsc16[:], dsc[:])

            # ---- constants / static data --------------------------------
            ident = cst.tile([P, P], f32)
            make_identity(nc, ident[:])
            W1s = cst.tile([P, P], f32)
            W2s = cst.tile([P, P], f32)
            iota = cst.tile([P, P], f16)
            nc.sync.dma_start(W1s[:], W1_t[:])
            nc.sync.dma_start(W2s[:], W2_t[:])
            nc.sync.dma_start(iota[:], iota_t[:])
            idx_sb = cst.tile([P, meta["idx_cols"]], mybir.dt.int16)
            nc.sync.dma_start(idx_sb[:], idx_t[:])
            slot_sb = cst.tile([P, NCH], f16)
            nc.sync.dma_start(slot_sb[:], slot_t[:])
            gm1s = cst.tile([P, 1], f32)
            bt1s = cst.tile([P, 1], f32)
            gm2s = cst.tile([P, 1], f32)
            bt2s = cst.tile([P, 1], f32)
            nc.sync.dma_start(gm1s[:], gm1[:])
            nc.sync.dma_start(bt1s[:], bt1[:])
            nc.sync.dma_start(gm2s[:], gm2[:])
            nc.sync.dma_start(bt2s[:], bt2[:])
            # fp16 copies of the weight matrices (mT is fp16-derived anyway)
            W1h = cst.tile([P, P], f16)
            W2h = cst.tile([P, P], f16)
            nc.vector.tensor_copy(W1h[:], W1s[:])
            nc.vector.tensor_copy(W2h[:], W2s[:])

            # d_in broadcast rows: din_bc[:, t*P+j] = d_in[j, t] for all rows
            din_bc = big.tile([P, T * P], f32)
            for t in range(T):
                bc_ps = ps.tile([P, P], f32, tag="tp")
                nc.tensor.transpose(
                    out=bc_ps[:],
                    in_=d_in[:, t : t + 1].to_broadcast([P, P]),
                    identity=ident[:],
                )
                nc.vector.tensor_copy(din_bc[:, t * P : (t + 1) * P], bc_ps[:])

            # persistent stores
            hpre = big.tile([P, T * P], f32)   # pre-BN activations [feat, dst]
            h16_shard = dram.tile([SHP, P], f16)
            h16_full = dram.tile([NN, P], f16, addr_space="Shared")

            gq = [0]

            def gconv_layer(table_full, W_sb, s1_cols, s2_cols, dsc_scale=None,
                            st_pre=None, mid_hook=None):
                """SpMM + W matmul; fills hpre and the per-tile stat columns.

                dsc_scale (layer 1): per-edge rsqrt(outdeg) image; gathered
                rows are scaled in place before the selector matmuls (the
                replicated x16 table is fed unnormalized).
                st_pre (layer 1): [P, T, P] f16 tile receiving transposed
                pre-BN hpre tiles as they finalize (hidden under the gather
                drain); BN+relu+d_out are applied post-AllReduce in the
                transposed layout, shortening the inter-layer critical path
                in front of the h16 AllGather."""
                srcA = table_full[0:SPLIT, :]
                srcB = table_full[SPLIT:NN, :]
                for ip, (t0, t1) in enumerate(pairs):
                    tl = [t0] if t0 == t1 else [t0, t1]
                    bufs = {}
                    for h, (Cc, offs, sv) in enumerate(
                        ((CA, meta["offs_a"], srcA), (CB, meta["offs_b"], srcB))
                    ):
                        col, wcols, nidx = offs[ip]
                        g = gat.tile([P, 2 * Cc, P], f16, tag=f"g{h}")
                        nch = nidx // P
                        step = max(1, GATHER_MAX_IDX // P)
                        for c0 in range(0, nch, step):
                            c1 = min(c0 + step, nch)
                            nc.gpsimd.dma_gather(
                                g[:, c0:c1, :],
                                sv,
                                idx_sb[:, col + c0 * 8 : col + c1 * 8],
                                (c1 - c0) * P,
                                (c1 - c0) * P,
                                P,
                                single_packet=GATHER_SINGLE_PACKET,
                                queue_num=gq[0] % N_SWDGE_QUEUES,
                            )
                            gq[0] += 1
                        if dsc_scale is not None:
                            offC = ip * 2 * (CA + CB) + (0 if h == 0 else 2 * CA)
                            nc.vector.tensor_tensor(
                                out=g[:],
                                in0=g[:],
                                in1=dsc_scale[:, offC : offC + 2 * Cc, None]
                                .to_broadcast([P, 2 * Cc, P]),
                                op=Alu.mult,
                            )
                        bufs[h] = g
                    NCHT = CA + CB
                    mTs_ps = []
                    sels = []
                    for ti, t in enumerate(tl):
                        cid0 = t * NCHT
                        sel = wrk.tile([P, NCHT, P], f16, tag="sel", bufs=4,
                                       name=f"sel{ti}")
                        nc.vector.tensor_tensor(
                            out=sel[:],
                            in0=slot_sb[:, cid0 : cid0 + NCHT][:, :, None]
                            .to_broadcast([P, NCHT, P]),
                            in1=iota[:, None, :].to_broadcast([P, NCHT, P]),
                            op=Alu.is_equal,
                        )
                        sels.append(sel)
                        mTs_ps.append(ps.tile([P, P], f32, tag="mT", bufs=4,
                                              name=f"mT{ti}"))
                    # interleave the two tiles' accumulation chains so the PE
                    # alternates PSUM banks (hides write-commit latency)
                    for c in range(NCHT):
                        h, cc = (0, c) if c < CA else (1, c - CA)
                        Cc = CA if h == 0 else CB
                        for ti in range(len(tl)):
                            nc.tensor.matmul(
                                out=mTs_ps[ti][:],
                                lhsT=bufs[h][:, ti * Cc + cc, :],
                                rhs=sels[ti][:, c, :],
                                start=(c == 0),
                                stop=(c == NCHT - 1),
                            )
                    for ti, t in enumerate(tl):
                        # evacuate with d_in column scaling (fp16 for the W mm)
                        mTs = wrk.tile([P, P], f16, tag="mTs")
                        nc.vector.tensor_tensor(
                            out=mTs[:],
                            in0=mTs_ps[ti][:],
                            in1=din_bc[:, t * P : (t + 1) * P],
                            op=Alu.mult,
                        )
                        hp = ps.tile([P, P], f32, tag="hp")
                        nc.tensor.matmul(
                            out=hp[:], lhsT=W_sb[:], rhs=mTs[:], start=True, stop=True
                        )
                        # evacuate + per-feature partial sums for BN
                        nc.vector.tensor_scalar(
                            hpre[:, t * P : (t + 1) * P],
                            hp[:],
                            1.0,
                            None,
                            Alu.mult,
                            Alu.add,
                            accum_out=s1_cols[:, t : t + 1],
                        )
                        sq = wrk.tile([P, P], f16, tag="sq")
                        nc.scalar.activation(
                            sq[:],
                            hpre[:, t * P : (t + 1) * P],
                            Act.Square,
                            accum_out=s2_cols[:, t : t + 1],
                        )
                        if st_pre is not None:
                            tpp = ps.tile([P, P], f32, tag="tp")
                            nc.tensor.transpose(
                                out=tpp[:],
                                in_=hpre[:, t * P : (t + 1) * P],
                                identity=ident[:],
                            )
                            nc.vector.tensor_copy(st_pre[:, t, :], tpp[:])
                    if mid_hook is not None and ip == max(0, len(pairs) - 5):
                        mid_hook()

            # The BN stats AllReduce is split in two: the bulk (tiles covered
            # by all but the last two pairs) fires from inside the gconv loop
            # and runs on the CC rings while the last pairs' gathers drain -
            # hiding both its transfer and the inter-core skew it absorbs.
            MIDCOL = pairs[max(0, len(pairs) - 5)][1] + 1

            def bn_ar(s1_cols, s2_cols, lo, hi, tag):
                """partial-sum AllReduce over stat columns [lo:hi)."""
                stats_in = dram.tile([P, 2], f32, name=f"stats_in_{tag}")
                stats_out = dram.tile(
                    [P, 2], f32, addr_space="Shared", name=f"stats_out_{tag}"
                )
                pack = wrk.tile([P, 2], f32, tag="pack")
                nc.vector.tensor_reduce(
                    pack[:, 0:1], s1_cols[:, lo:hi],
                    axis=mybir.AxisListType.X, op=Alu.add,
                )
                nc.vector.tensor_reduce(
                    pack[:, 1:2], s2_cols[:, lo:hi],
                    axis=mybir.AxisListType.X, op=Alu.add,
                )
                nc.sync.dma_start(stats_in[:], pack[:])
                nc.gpsimd.collective_compute(
                    "AllReduce",
                    Alu.add,
                    replica_groups=[list(range(N_CORES))],
                    ins=[stats_in.opt()],
                    outs=[stats_out.opt()],
                )
                return stats_out

            def bn_coeffs(stats1, stats2, gam, bet):
                """merge the two AllReduced stat halves -> scale a, shift c."""
                glob = wrk.tile([P, 2], f32, tag="glob")
                nc.sync.dma_start(glob[:], stats1[:])
                g2 = wrk.tile([P, 2], f32, tag="glob2")
                nc.sync.dma_start(g2[:], stats2[:])
                nc.vector.tensor_tensor(
                    out=glob[:], in0=glob[:], in1=g2[:], op=Alu.add
                )
                mo = wrk.tile([P, 4], f32, tag="mo")
                # mo: 0=mu 1=E[h^2] 2=var+eps 3=scratch
                nc.vector.tensor_scalar(mo[:, 0:2], glob[:], 1.0 / NN, None, Alu.mult)
                nc.vector.tensor_tensor(
                    out=mo[:, 3:4], in0=mo[:, 0:1], in1=mo[:, 0:1], op=Alu.mult
                )
                nc.vector.tensor_tensor(
                    out=mo[:, 2:3], in0=mo[:, 1:2], in1=mo[:, 3:4], op=Alu.subtract
                )
                nc.vector.tensor_scalar_add(mo[:, 2:3], mo[:, 2:3], EPS)
                nc.scalar.sqrt(mo[:, 2:3], mo[:, 2:3])
                a_c = cst.tile([P, 2], f32, name=f"a_c_{gam.name}")
                nc.vector.reciprocal(a_c[:, 0:1], mo[:, 2:3])
                nc.vector.tensor_tensor(
                    out=a_c[:, 0:1], in0=a_c[:, 0:1], in1=gam[:], op=Alu.mult
                )
                nc.vector.tensor_tensor(
                    out=a_c[:, 1:2], in0=a_c[:, 0:1], in1=mo[:, 0:1], op=Alu.mult
                )
                nc.vector.tensor_tensor(
                    out=a_c[:, 1:2], in0=bet[:], in1=a_c[:, 1:2], op=Alu.subtract
                )
                return a_c

            # ================= layer 1 =================
            s1a = cst.tile([P, T], f32)
            s2a = cst.tile([P, T], f32)
            st_all = big.tile([P, T, P], f16, tag="big16b")
            ar1 = []
            gconv_layer(
                x16_full, W1h, s1a, s2a, dsc_scale=dsc16, st_pre=st_all,
                mid_hook=lambda: ar1.append(bn_ar(s1a, s2a, 0, MIDCOL, "l1a")),
            )
            ar1b = bn_ar(s1a, s2a, MIDCOL, T, "l1b")
            ac1 = bn_coeffs(ar1[0], ar1b, gm1s, bt1s)

            # h16 table: BN+relu+d_out applied to the pre-transposed st_all
            # in place (a/c broadcast along the free feat dim via row tiles)
            a_bc = cst.tile([P, P], f16, name="a_bc")
            c_bc = cst.tile([P, P], f16, name="c_bc")
            for colv, bc in ((ac1[:, 0:1], a_bc), (ac1[:, 1:2], c_bc)):
                bc_ps = ps.tile([P, P], f32, tag="tp")
                nc.tensor.transpose(
                    out=bc_ps[:],
                    in_=colv.to_broadcast([P, P]),
                    identity=ident[:],
                )
                nc.vector.tensor_copy(bc[:], bc_ps[:])
            d_out16 = cst.tile([P, T], f16, name="d_out16")
            nc.vector.tensor_copy(d_out16[:], d_out[:])
            nc.vector.tensor_tensor(
                out=st_all[:], in0=st_all[:],
                in1=a_bc[:, None, :].to_broadcast([P, T, P]), op=Alu.mult,
            )
            nc.vector.tensor_tensor(
                out=st_all[:], in0=st_all[:],
                in1=c_bc[:, None, :].to_broadcast([P, T, P]), op=Alu.add,
            )
            nc.vector.tensor_scalar(st_all[:], st_all[:], 0.0, None, Alu.max)
            nc.vector.tensor_tensor(
                out=st_all[:], in0=st_all[:],
                in1=d_out16[:, :, None].to_broadcast([P, T, P]), op=Alu.mult,
            )
            nc.sync.dma_start(
                h16_shard.rearrange("(t p) f -> p t f", p=P), st_all[:]
            )
            nc.gpsimd.collective_compute(
                "AllGather",
                Alu.bypass,
                replica_groups=[list(range(N_CORES))],
                ins=[h16_shard[0:SH, :].opt()],
                outs=[h16_full.opt()],
            )

            # ================= layer 2 =================
            # pre-BN hpre2 tiles are transposed into oall during the layer-2
            # drain; the tail then applies BN2 + residual + relu entirely in
            # the transposed layout. The residual h1^T is reconstructed from
            # the still-live h16 table: st_all = relu(bn(h1pre))*d_out, so
            # h1^T = st_all * (1/d_out).
            s1b = cst.tile([P, T], f32)
            s2b = cst.tile([P, T], f32)
            oall = big.tile([P, T, P], f32, tag="bigf32")
            ar2 = []
            gconv_layer(
                h16_full, W2h, s1b, s2b, st_pre=oall,
                mid_hook=lambda: ar2.append(bn_ar(s1b, s2b, 0, MIDCOL, "l2a")),
            )
            ar2b = bn_ar(s1b, s2b, MIDCOL, T, "l2b")
            ac2 = bn_coeffs(ar2[0], ar2b, gm2s, bt2s)

            a2_bc = cst.tile([P, P], f32, name="a2_bc")
            c2_bc = cst.tile([P, P], f32, name="c2_bc")
            for colv, bc in ((ac2[:, 0:1], a2_bc), (ac2[:, 1:2], c2_bc)):
                bc_ps = ps.tile([P, P], f32, tag="tp")
                nc.tensor.transpose(
                    out=bc_ps[:],
                    in_=colv.to_broadcast([P, P]),
                    identity=ident[:],
                )
                nc.vector.tensor_copy(bc[:], bc_ps[:])
            douti = wrk.tile([P, T], f32, tag="degraw")
            nc.vector.reciprocal(douti[:], d_out[:])
            douti16 = cst.tile([P, T], f16, name="douti16")
            nc.vector.tensor_copy(douti16[:], douti[:])

            nc.vector.tensor_tensor(
                out=oall[:], in0=oall[:],
                in1=a2_bc[:, None, :].to_broadcast([P, T, P]), op=Alu.mult,
            )
            nc.vector.tensor_tensor(
                out=oall[:], in0=oall[:],
                in1=c2_bc[:, None, :].to_broadcast([P, T, P]), op=Alu.add,
            )
            nc.vector.tensor_tensor(
                out=st_all[:], in0=st_all[:],
                in1=douti16[:, :, None].to_broadcast([P, T, P]), op=Alu.mult,
            )
            nc.vector.tensor_tensor(
                out=oall[:], in0=oall[:], in1=st_all[:], op=Alu.add,
            )
            nc.vector.tensor_scalar(oall[:], oall[:], 0.0, None, Alu.max)
            nc.sync.dma_start(out_t.rearrange("(t p) f -> p t f", p=P), oall[:])

    nc.compile()
    return nc


# ---------------------------------------------------------------------------


_CACHE = {}


def _get_program(meta):
    key = (meta["SH"], meta["T"], meta["CA"], meta["CB"], meta["idx_cols"])
    if key not in _CACHE:
        _CACHE[key] = _build_program(meta)
    return _CACHE[key]


def _build_in_maps(meta, cores, inputs):
    x = np.asarray(inputs["x"], np.float32)
    SH, T = meta["SH"], meta["T"]
    SHP = T * P
    iota = np.tile(np.arange(P).astype(_NP16), (P, 1))
    # layer-1 gather table: replicated narrow-float copy of x (pure dtype
    # conversion; the d_out norm is applied on device to gathered rows)
    x16_rep = np.ascontiguousarray(x.astype(_NP16))
    in_maps = []
    for k in range(N_CORES):
        c = cores[k]
        in_maps.append(
            {
                "x16_rep": x16_rep,
                "W1": np.asarray(inputs["W1"], np.float32),
                "W2": np.asarray(inputs["W2"], np.float32),
                "gamma1": np.asarray(inputs["gamma1"], np.float32).reshape(P, 1),
                "beta1": np.asarray(inputs["beta1"], np.float32).reshape(P, 1),
                "gamma2": np.asarray(inputs["gamma2"], np.float32).reshape(P, 1),
                "beta2": np.asarray(inputs["beta2"], np.float32).reshape(P, 1),
                "iota": iota,
                "idx_img": c["idx_img"],
                "slotT": c["slotT"],
                "degT": c["degT"],
                "deg_out": c["deg_out"],
                "deg_in": c["deg_in"],
            }
        )
    return in_maps


def kernel(**inputs):
    x = np.asarray(inputs["x"], np.float32)
    src = np.asarray(inputs["src"])
    dst = np.asarray(inputs["dst"])
    n_nodes = x.shape[0]

    meta, cores = _host_prep(src, dst, n_nodes)
    nc = _get_program(meta)
    in_maps = _build_in_maps(meta, cores, inputs)

    from concourse.bass_utils import run_bass_kernel_spmd

    res = run_bass_kernel_spmd(nc, in_maps, core_ids=list(range(N_CORES)))
    SH = meta["SH"]
    out = np.concatenate(
        [res.results[k]["out"][:SH] for k in range(N_CORES)], axis=0
    )
    return out.astype(np.float32)

